# revision 19
# baseline (speedup 1.0000x reference)
"""AudioStructuralAnalyzer Trainium2 kernel.

Sharding: pure data parallel - batch item k -> NeuronCore k (8 batches, 8 cores).
Work is split into 4 column chunks of 512 (+4 halo each side); each chunk is a
separate bass_exec dispatch so input upload, execution, and output download
pipeline on the (slow, ~35 MB/s) axon tunnel.

Per core per chunk: input [2, 256, 520] f16, output [6, 256, 512] uint8 (x254).

Per-channel pipeline (validated against the jax reference in fp32 numpy):
  H-direction conv parts  -> PE banded matmuls (float32r, 1 cyc/col)
  W-direction conv taps   -> DVE shifted-AP tensor ops
  transcendentals         -> ACT (Sqrt/Square/Ln/Abs), reciprocal via DVE approx
Entropy uses the z = disc/trace form:  ent = 1 - [(1+z)ln(1+z)+(1-z)ln(1-z)]/(2 ln2).

Wire format is f16 in / uint8 out to minimize tunnel traffic: the ACT f32->u8
conversion rounds-to-nearest and saturates, and all six maps live in [0, 1],
so a fixed x254 scale loses <0.2% relL2.  A per-chunk mask input zeroes the
gradient-stage halo columns at the true image edges (emulating the reference's
zero padding of the intermediate maps); interior chunk borders keep the real
neighbor values.  Output operands of the custom call are never read by the
NEFF (their input binding is dropped by the tensor rename), so device-resident
"ghost" zeros are passed once and reused; results land in fresh PJRT buffers
that the kernel fully overwrites.
"""
import numpy as np
import jax
import jax.numpy as jnp
from jax.sharding import Mesh, PartitionSpec as P, NamedSharding
from concurrent.futures import ThreadPoolExecutor

try:
    from jax import shard_map as _jax_shard_map

    def _shard_map(f, **kw):
        kw["check_vma"] = kw.pop("check_rep")
        return _jax_shard_map(f, **kw)
except ImportError:
    from jax.experimental.shard_map import shard_map as _shard_map

import concourse.bass as bass
import concourse.tile as tile
import concourse.mybir as mybir
from concourse import bacc
import concourse.bass2jax as b2j

F32 = mybir.dt.float32
F16 = mybir.dt.float16
U8 = mybir.dt.uint8
AF = mybir.ActivationFunctionType
OP = mybir.AluOpType

import os as _os

_DEBUG = bool(_os.environ.get("ASA_DEBUG"))

EPS = 1e-10
H, Wimg = 256, 2048
S = 512          # chunk output width
PAD = 4          # chunk halo
W = S + 2 * PAD  # chunk input width
NCHUNKS = Wimg // S
N_CORES = 8
OSCALE = 254.0   # uint8 wire scale for the [0,1] output maps


def _band(taps, c):
    """B[k, m] = taps[d] where k = m + d - c  (correlation, zero pad)."""
    B = np.zeros((H, H), np.float32)
    for d, w in enumerate(taps):
        off = d - c
        ks = np.arange(max(0, off), min(H, H + off))
        B[ks, ks - off] = np.float32(w)
    return B


def _build_program(g1, sxh, syh, harm_taps):
    """g1: 5-tap gaussian factor (sums to 1); sxh/syh: 3-tap H parts of the
    sobels (already /8); harm_taps: 7-tap harmonic H filter."""
    a, b, c0 = float(g1[0]), float(g1[1]), float(g1[2])
    s_ab, s_bc = a / b, b / c0

    bands_np = {
        "b3s": _band(sxh, 1),
        "b3d": _band(syh, 1),
        "bh": _band(harm_taps, 3),
        "bg5": _band(g1, 2) * np.float32(c0),
        "bg5h": _band(g1, 2) * np.float32(0.5 * c0),
    }

    nc = bacc.Bacc("TRN2", target_bir_lowering=False, debug=False)
    x_d = nc.declare_dram_parameter("x", [2, H, W], F16, isOutput=False)
    band_d = {k: nc.declare_dram_parameter(k, [H, H], F32, isOutput=False)
              for k in bands_np}
    mask_d = nc.declare_dram_parameter("mask", [128, 8], F32, isOutput=False)
    out_names = ["ent", "al", "cur", "harm", "tmp", "spec"]
    out_d = nc.declare_dram_parameter("out", [len(out_names), H, S], U8,
                                      isOutput=True)

    with tile.TileContext(nc) as tc:
        with (
            tc.tile_pool(name="bands", bufs=1) as bp,
            tc.tile_pool(name="sb", bufs=1) as sb,
            tc.tile_pool(name="ps", bufs=4, space="PSUM") as pp,
        ):
            band_t = {}
            for k in bands_np:
                band_t[k] = [bp.tile([128, H], F32, tag=f"{k}{j}", name=f"{k}{j}") for j in (0, 1)]
                for j in (0, 1):
                    nc.sync.dma_start(band_t[k][j][:], band_d[k][j * 128:(j + 1) * 128, :])
            mask_t = bp.tile([128, 8], F32, tag="mask", name="mask")
            nc.sync.dma_start(mask_t[:], mask_d[:, :])

            cEPS = bp.tile([128, 1], F32, tag="cEPS", name="cEPS")
            nc.vector.memset(cEPS[:], EPS)
            cONE = bp.tile([128, 1], F32, tag="cONE", name="cONE")
            nc.vector.memset(cONE[:], 1.0)
            cTINY = bp.tile([128, 1], F32, tag="cTINY", name="cTINY")
            nc.vector.memset(cTINY[:], 1e-30)

            def pair(tag, dt=F32, width=W):
                return [sb.tile([128, width], dt, tag=f"{tag}{j}", name=f"{tag}{j}")
                        for j in (0, 1)]

            def tt(outp, ap0, ap1, op, lo, hi):
                for j in (0, 1):
                    nc.vector.tensor_tensor(out=outp[j][:, lo:hi], in0=ap0[j],
                                            in1=ap1[j], op=op)

            def act(outp, inp, func, lo, hi, bias=None, scale=1.0):
                for j in (0, 1):
                    nc.scalar.activation(outp[j][:, lo:hi], inp[j], func,
                                         bias=(bias[:] if bias is not None else 0.0),
                                         scale=scale)

            def hconv(bname, xpair, tag):
                """PE banded H-conv: returns PSUM tile pair."""
                B = bands_np[bname]
                outs = []
                for m in (0, 1):
                    o = pp.tile([128, W], F32, tag="ps", name=f"ps_{tag}{m}")
                    ks = [k for k in (0, 1)
                          if np.abs(B[k * 128:(k + 1) * 128,
                                      m * 128:(m + 1) * 128]).max() > 0]
                    for c0_, c1_ in ((0, 256), (256, 512), (512, W)):
                        for i, k in enumerate(ks):
                            nc.tensor.matmul(
                                o[:, c0_:c1_],
                                band_t[bname][k][:, m * 128:(m + 1) * 128],
                                xpair[k][:, c0_:c1_],
                                start=(i == 0), stop=(i == len(ks) - 1))
                    outs.append(o)
                return outs

            def g5w(inp, tag, lo=3, hi=W - 3):
                """5-tap gaussian W-conv (divided by center weight c0):
                valid out cols [3, W-3). Reads inp cols [1, W-1)."""
                t1, t2, s1 = pair("g5t1"), pair("g5t2"), pair("g5s1")
                o = pair("g5wf")
                for j in (0, 1):
                    nc.vector.tensor_add(t1[j][:, lo:hi], inp[j][:, lo - 2:hi - 2],
                                         inp[j][:, lo + 2:hi + 2])
                    nc.vector.tensor_add(t2[j][:, lo:hi], inp[j][:, lo - 1:hi - 1],
                                         inp[j][:, lo + 1:hi + 1])
                    nc.vector.scalar_tensor_tensor(
                        out=s1[j][:, lo:hi], in0=t1[j][:, lo:hi], scalar=s_ab,
                        in1=t2[j][:, lo:hi], op0=OP.mult, op1=OP.add)
                    nc.vector.scalar_tensor_tensor(
                        out=o[j][:, lo:hi], in0=s1[j][:, lo:hi], scalar=s_bc,
                        in1=inp[j][:, lo:hi], op0=OP.mult, op1=OP.add)
                return o

            def mask_edges(tpair):
                """Multiply the gradient-stage halo cols by the edge mask:
                x0 at a true image edge (emulates the reference's zero pad of
                the intermediate maps), x1 at interior chunk borders.  Cols 0
                and W-1 are never computed nor read downstream in a way that
                propagates, so only [1,PAD) and [W-PAD,W-1) need masking."""
                for j in (0, 1):
                    nc.vector.tensor_tensor(
                        out=tpair[j][:, 1:PAD], in0=tpair[j][:, 1:PAD],
                        in1=mask_t[:, 1:PAD], op=OP.mult)
                    nc.vector.tensor_tensor(
                        out=tpair[j][:, W - PAD:W - 1],
                        in0=tpair[j][:, W - PAD:W - 1],
                        in1=mask_t[:, 4:7], op=OP.mult)

            keep = {}
            for ch in (0, 1):
                xh = pair("xh", F16)
                x = pair("x")
                for j in (0, 1):
                    nc.sync.dma_start(xh[j][:],
                                      x_d[ch, j * 128:(j + 1) * 128, 0:W])
                act(x, [xh[j][:, 0:W] for j in (0, 1)], AF.Copy, 0, W)
                # ---- phase A: sobel/harmonic H-parts on PE ----
                sx = hconv("b3s", x, "sx")
                sx_s = pair("q1")
                act(sx_s, [sx[j][:, 0:W] for j in (0, 1)], AF.Copy, 0, W)
                gte = pair("gte")
                for j in (0, 1):
                    nc.vector.scalar_tensor_tensor(
                        out=gte[j][:, 1:W - 1], in0=sx_s[j][:, 2:W], scalar=EPS,
                        in1=sx_s[j][:, 0:W - 2], op0=OP.add, op1=OP.subtract)
                sy = hconv("b3d", x, "sy")
                sy_s = pair("q2")
                act(sy_s, [sy[j][:, 0:W] for j in (0, 1)], AF.Copy, 0, W)
                tsc = pair("tsc")
                gf = pair("gf")
                for j in (0, 1):
                    nc.vector.tensor_add(tsc[j][:, 0:W - 1], sy_s[j][:, 0:W - 1],
                                         sy_s[j][:, 1:W])
                    nc.vector.tensor_add(gf[j][:, 1:W - 1], tsc[j][:, 0:W - 2],
                                         tsc[j][:, 1:W - 1])
                hp = hconv("bh", x, "hp")
                ha = pair("ha")
                for j in (0, 1):
                    nc.scalar.activation(ha[j][:, 0:W], hp[j][:, 0:W], AF.Abs)
                # ---- phase B: pointwise gradient stage ----
                xsq = pair("xsq")
                act(xsq, [x[j][:, 0:W] for j in (0, 1)], AF.Square, 0, W)
                q1, q2 = pair("q1"), pair("q2")
                act(q1, [gte[j][:, 1:W - 1] for j in (0, 1)], AF.Square, 1, W - 1)
                act(q2, [gf[j][:, 1:W - 1] for j in (0, 1)], AF.Square, 1, W - 1)
                h2, Dp, Pp = pair("h2"), pair("Dp"), pair("Pp")
                tt(h2, [q1[j][:, 1:W - 1] for j in (0, 1)],
                   [q2[j][:, 1:W - 1] for j in (0, 1)], OP.add, 1, W - 1)
                tt(Dp, [q1[j][:, 1:W - 1] for j in (0, 1)],
                   [q2[j][:, 1:W - 1] for j in (0, 1)], OP.subtract, 1, W - 1)
                tt(Pp, [gte[j][:, 1:W - 1] for j in (0, 1)],
                   [gf[j][:, 1:W - 1] for j in (0, 1)], OP.mult, 1, W - 1)
                hmag, inv = pair("hmag"), pair("inv")
                act(hmag, [h2[j][:, 1:W - 1] for j in (0, 1)], AF.Sqrt,
                    1, W - 1, bias=cTINY)
                for j in (0, 1):
                    nc.vector.reciprocal_approx_fast(out=inv[j][:, 1:W - 1],
                                                     in_=hmag[j][:, 1:W - 1])
                ux, uy, gfa = pair("ux"), pair("uy"), pair("gfa")
                tt(ux, [gte[j][:, 1:W - 1] for j in (0, 1)],
                   [inv[j][:, 1:W - 1] for j in (0, 1)], OP.mult, 1, W - 1)
                tt(uy, [gf[j][:, 1:W - 1] for j in (0, 1)],
                   [inv[j][:, 1:W - 1] for j in (0, 1)], OP.mult, 1, W - 1)
                act(gfa, [gf[j][:, 1:W - 1] for j in (0, 1)], AF.Abs, 1, W - 1)
                mask_edges(ux)
                mask_edges(uy)
                mask_edges(gfa)
                # ---- phase C/D: the seven G5s (W-part DVE, H-part PE) ----
                def g5full(inp, tag):
                    wf = g5w(inp, tag)
                    return hconv("bg5", wf, f"g5_{tag}")

                tr_ps = g5full(h2, "h2")
                tr = pair("tr")
                act(tr, [tr_ps[j][:, 3:W - 3] for j in (0, 1)], AF.Copy, 3, W - 3)
                df_ps = g5full(Dp, "Dp")
                e1 = pair("q1")
                act(e1, [df_ps[j][:, 3:W - 3] for j in (0, 1)], AF.Square, 3, W - 3)
                ps_ps = g5full(Pp, "Pp")
                e2 = pair("q2")
                act(e2, [ps_ps[j][:, 3:W - 3] for j in (0, 1)], AF.Square,
                    3, W - 3, scale=2.0)
                dsq, disc, trr, z = pair("tsc"), pair("hmag"), pair("inv"), pair("h2")
                tt(dsq, [e1[j][:, 3:W - 3] for j in (0, 1)],
                   [e2[j][:, 3:W - 3] for j in (0, 1)], OP.add, 3, W - 3)
                act(disc, [dsq[j][:, 3:W - 3] for j in (0, 1)], AF.Sqrt,
                    3, W - 3, bias=cEPS)
                for j in (0, 1):
                    nc.vector.reciprocal_approx_fast(out=trr[j][:, 3:W - 3],
                                                     in_=tr[j][:, 3:W - 3])
                tt(z, [disc[j][:, 3:W - 3] for j in (0, 1)],
                   [trr[j][:, 3:W - 3] for j in (0, 1)], OP.mult, 3, W - 3)
                zc, lu, lv, wt, w2, ee = (pair("Dp"), pair("Pp"), pair("lv"),
                                          pair("q1"), pair("q2"), pair("tsc"))
                for j in (0, 1):
                    nc.vector.tensor_scalar(
                        out=zc[j][:, 3:W - 3], in0=z[j][:, 3:W - 3],
                        scalar1=0.99999988, scalar2=0.0, op0=OP.min, op1=OP.max)
                act(lu, [zc[j][:, 3:W - 3] for j in (0, 1)], AF.Ln, 3, W - 3,
                    bias=cONE)
                act(lv, [zc[j][:, 3:W - 3] for j in (0, 1)], AF.Ln, 3, W - 3,
                    bias=cONE, scale=-1.0)
                for j in (0, 1):
                    nc.vector.scalar_tensor_tensor(
                        out=wt[j][:, 3:W - 3], in0=zc[j][:, 3:W - 3], scalar=1.0,
                        in1=lu[j][:, 3:W - 3], op0=OP.add, op1=OP.mult)
                    nc.vector.scalar_tensor_tensor(
                        out=w2[j][:, 3:W - 3], in0=zc[j][:, 3:W - 3], scalar=1.0,
                        in1=lv[j][:, 3:W - 3], op0=OP.subtract, op1=OP.mult)
                tt(ee, [wt[j][:, 3:W - 3] for j in (0, 1)],
                   [w2[j][:, 3:W - 3] for j in (0, 1)], OP.subtract, 3, W - 3)
                enth = pair(f"enth{ch}")
                for j in (0, 1):
                    nc.vector.tensor_scalar(
                        out=enth[j][:, 3:W - 3], in0=ee[j][:, 3:W - 3],
                        scalar1=-0.36067376, scalar2=0.5, op0=OP.mult, op1=OP.add)
                # alignment
                ux_ps = g5full(ux, "ux")
                a1 = pair("q1")
                act(a1, [ux_ps[j][:, 3:W - 3] for j in (0, 1)], AF.Square, 3, W - 3)
                uy_ps = g5full(uy, "uy")
                a2 = pair("q2")
                act(a2, [uy_ps[j][:, 3:W - 3] for j in (0, 1)], AF.Square, 3, W - 3)
                qs, alv = pair("h2"), pair("hmag")
                tt(qs, [a1[j][:, 3:W - 3] for j in (0, 1)],
                   [a2[j][:, 3:W - 3] for j in (0, 1)], OP.add, 3, W - 3)
                act(alv, [qs[j][:, 3:W - 3] for j in (0, 1)], AF.Sqrt, 3, W - 3,
                    bias=cEPS)
                alh = pair(f"alh{ch}")
                for j in (0, 1):
                    nc.vector.tensor_scalar(
                        out=alh[j][:, 3:W - 3], in0=alv[j][:, 3:W - 3],
                        scalar1=1.0, scalar2=0.5, op0=OP.min, op1=OP.mult)
                # harmonic
                le_ps = g5full(xsq, "xsq")
                le_s, rle, hrr = pair("Dp"), pair("Pp"), pair("h2")
                act(le_s, [le_ps[j][:, 3:W - 3] for j in (0, 1)], AF.Copy, 3, W - 3)
                for j in (0, 1):
                    nc.vector.reciprocal_approx_fast(out=rle[j][:, 3:W - 3],
                                                     in_=le_s[j][:, 3:W - 3])
                tt(hrr, [ha[j][:, 3:W - 3] for j in (0, 1)],
                   [rle[j][:, 3:W - 3] for j in (0, 1)], OP.mult, 3, W - 3)
                hc = pair(f"hc{ch}")
                for j in (0, 1):
                    nc.vector.tensor_scalar(
                        out=hc[j][:, 3:W - 3], in0=hrr[j][:, 3:W - 3],
                        scalar1=1.0, scalar2=0.5, op0=OP.min, op1=OP.mult)
                # spectral (per channel, clip active)
                sp_ps = g5full(gfa, "gfa")
                spc = pair(f"spc{ch}")
                for j in (0, 1):
                    nc.vector.tensor_scalar(
                        out=spc[j][:, 3:W - 3], in0=sp_ps[j][:, 3:W - 3],
                        scalar1=1.0, scalar2=0.5, op0=OP.min, op1=OP.mult)
                # curvature (per-channel curv; G5 after the channel mean)
                dudx, dvdx = pair("q1"), pair("q2")
                for src_u, dst in ((ux, dudx), (uy, dvdx)):
                    axp = hconv("b3s", src_u, "ax")
                    axs = pair("g5s1")
                    act(axs, [axp[j][:, 1:W - 1] for j in (0, 1)], AF.Copy,
                        1, W - 1)
                    for j in (0, 1):
                        nc.vector.tensor_sub(dst[j][:, 2:W - 2],
                                             axs[j][:, 3:W - 1],
                                             axs[j][:, 1:W - 3])
                dudy, dvdy = pair("tsc"), pair("hmag")
                for nm, src_u, dst in (("g5t1", ux, dudy), ("g5t2", uy, dvdy)):
                    bxp = hconv("b3d", src_u, "bx")
                    bxs = pair("g5wf")
                    act(bxs, [bxp[j][:, 0:W] for j in (0, 1)], AF.Copy, 0, W)
                    tpw = pair(nm)
                    for j in (0, 1):
                        nc.vector.tensor_add(tpw[j][:, 1:W - 1], bxs[j][:, 1:W - 1],
                                             bxs[j][:, 2:W])
                        nc.vector.tensor_add(dst[j][:, 2:W - 2], tpw[j][:, 1:W - 3],
                                             tpw[j][:, 2:W - 2])
                c1_, c2_, c3_, c4_ = pair("Dp"), pair("Pp"), pair("h2"), pair("lv")
                act(c1_, [dudx[j][:, 2:W - 2] for j in (0, 1)], AF.Square, 2, W - 2)
                act(c2_, [dudy[j][:, 2:W - 2] for j in (0, 1)], AF.Square, 2, W - 2)
                act(c3_, [dvdx[j][:, 2:W - 2] for j in (0, 1)], AF.Square, 2, W - 2)
                act(c4_, [dvdy[j][:, 2:W - 2] for j in (0, 1)], AF.Square, 2, W - 2)
                ss1, ss2, ss3 = pair("q1"), pair("q2"), pair("g5t1")
                tt(ss1, [c1_[j][:, 2:W - 2] for j in (0, 1)],
                   [c2_[j][:, 2:W - 2] for j in (0, 1)], OP.add, 2, W - 2)
                tt(ss2, [c3_[j][:, 2:W - 2] for j in (0, 1)],
                   [c4_[j][:, 2:W - 2] for j in (0, 1)], OP.add, 2, W - 2)
                tt(ss3, [ss1[j][:, 2:W - 2] for j in (0, 1)],
                   [ss2[j][:, 2:W - 2] for j in (0, 1)], OP.add, 2, W - 2)
                curv = pair(f"curv{ch}")
                act(curv, [ss3[j][:, 2:W - 2] for j in (0, 1)], AF.Sqrt,
                    2, W - 2, bias=cEPS)
                # temporal
                tb = pair("hmag")
                act(tb, [gte[j][:, 1:W - 1] for j in (0, 1)], AF.Abs, 1, W - 1)
                tb1s = pair("Dp")
                for j in (0, 1):
                    nc.vector.tensor_scalar_add(tb1s[j][:, 1:W - 1],
                                                tb[j][:, 1:W - 1], 1.0)
                rtc = pair(f"rt{ch}")
                for j in (0, 1):
                    nc.vector.reciprocal_approx_fast(out=rtc[j][:, 1:W - 1],
                                                     in_=tb1s[j][:, 1:W - 1])
                mask_edges(rtc)
                keep[ch] = dict(enth=enth, alh=alh, hc=hc, spc=spc, curv=curv,
                                rt=rtc)
            # ---- combine channels ----
            stage = {}
            for nm, key in (("ent", "enth"), ("al", "alh"), ("harm", "hc"),
                            ("spec", "spc")):
                o = pair(f"st_{nm}")
                tt(o, [keep[0][key][j][:, 3:W - 3] for j in (0, 1)],
                   [keep[1][key][j][:, 3:W - 3] for j in (0, 1)], OP.add, 3, W - 3)
                stage[nm] = o
            curv_m, tmp_m = pair("curv_m"), pair("tmp_m")
            tt(curv_m, [keep[0]["curv"][j][:, 2:W - 2] for j in (0, 1)],
               [keep[1]["curv"][j][:, 2:W - 2] for j in (0, 1)], OP.add, 2, W - 2)
            tt(tmp_m, [keep[0]["rt"][j][:, 1:W - 1] for j in (0, 1)],
               [keep[1]["rt"][j][:, 1:W - 1] for j in (0, 1)], OP.add, 1, W - 1)
            wf_cv = g5w(curv_m, "cvm", lo=4, hi=W - 4)
            cv_ps = hconv("bg5h", wf_cv, "cvf")
            o = pair("st_cur")
            act(o, [cv_ps[j][:, 4:W - 4] for j in (0, 1)], AF.Copy, 4, W - 4)
            stage["cur"] = o
            wf_tm = g5w(tmp_m, "tmm", lo=3, hi=W - 3)
            tm_ps = hconv("bg5h", wf_tm, "tmf")
            o = pair("st_tmp")
            act(o, [tm_ps[j][:, 3:W - 3] for j in (0, 1)], AF.Copy, 3, W - 3)
            stage["tmp"] = o
            # ---- uint8 wire conversion + store ----
            for oi, nm in enumerate(out_names):
                u8 = pair(f"u8_{nm}", U8, S)
                act(u8, [stage[nm][j][:, PAD:PAD + S] for j in (0, 1)],
                    AF.Copy, 0, S, scale=OSCALE)
                for j in (0, 1):
                    nc.sync.dma_start(
                        out_d[oi, j * 128:(j + 1) * 128, 0:S],
                        u8[j][:, 0:S])

    nc.finalize()
    return nc, out_names


class _Runtime:
    pass


def _setup(g1, sxh, syh, hk, dev_lo=0, dev_hi=N_CORES):
    nc, out_names = _build_program(g1, sxh, syh, hk)

    b2j.install_neuronx_cc_hook()

    partition_name = None
    pt = getattr(nc, "partition_id_tensor", None)
    if pt is not None:
        partition_name = pt.name

    in_names = []
    out_names_bir = []
    out_avals = []
    for alloc in nc.m.functions[0].allocations:
        if not isinstance(alloc, mybir.MemoryLocationSet):
            continue
        name = alloc.memorylocations[0].name
        if alloc.kind == "ExternalInput":
            if name != partition_name:
                in_names.append(name)
        elif alloc.kind == "ExternalOutput":
            out_names_bir.append(name)
            out_avals.append(jax.core.ShapedArray(tuple(alloc.tensor_shape),
                                                  mybir.dt.np(alloc.dtype)))
    assert in_names[0] == "x", in_names
    assert in_names[-1] == "mask", in_names

    bind_in_names = tuple(in_names) + tuple(out_names_bir)
    if partition_name is not None:
        bind_in_names = bind_in_names + (partition_name,)

    devices = jax.devices()[dev_lo:dev_hi]
    n_local = len(devices)
    mesh = Mesh(np.asarray(devices), ("core",))
    n_repl = len(in_names) - 1   # bands + mask, replicated
    n_outs = len(out_names_bir)

    def _body(*ops):
        operands = list(ops)
        if partition_name is not None:
            operands.append(b2j.partition_id_tensor())
        outs = b2j._bass_exec_p.bind(
            *operands,
            out_avals=tuple(out_avals),
            in_names=bind_in_names,
            out_names=tuple(out_names_bir),
            lowering_input_output_aliases=(),
            sim_require_finite=True,
            sim_require_nnan=True,
            nc=nc,
        )
        return tuple(outs)

    in_specs = (P("core"),) + (P(),) * n_repl + (P("core"),) * n_outs
    out_specs = (P("core"),) * n_outs
    sharded = jax.jit(_shard_map(_body, mesh=mesh, in_specs=in_specs,
                                 out_specs=out_specs, check_rep=False),
                      keep_unused=True)

    a, b, c0 = float(g1[0]), float(g1[1]), float(g1[2])
    bands = {
        "b3s": _band(sxh, 1),
        "b3d": _band(syh, 1),
        "bh": _band(hk, 3),
        "bg5": _band(g1, 2) * np.float32(c0),
        "bg5h": _band(g1, 2) * np.float32(0.5 * c0),
    }

    rt = _Runtime()
    rt.sharded = sharded
    rt.n_local = n_local
    rt.x_sharding = NamedSharding(mesh, P("core"))
    repl = NamedSharding(mesh, P())
    rt.band_dev = [jax.device_put(bands[nm], repl) for nm in in_names[1:-1]]
    masks = {}
    for kind, (ml, mr) in (("left", (0.0, 1.0)), ("mid", (1.0, 1.0)),
                           ("right", (1.0, 0.0))):
        m = np.empty((128, 8), np.float32)
        m[:, :4] = ml
        m[:, 4:] = mr
        masks[kind] = jax.device_put(m, repl)
    rt.mask_dev = [masks["left"]] + [masks["mid"]] * (NCHUNKS - 2) + [masks["right"]]
    out_sharding = NamedSharding(mesh, P("core"))
    rt.ghosts = [jax.device_put(
        np.zeros((n_local * av.shape[0], *av.shape[1:]), av.dtype), out_sharding)
        for av in out_avals]
    jax.block_until_ready(rt.band_dev)
    jax.block_until_ready(rt.ghosts)
    jax.block_until_ready(list(masks.values()))
    return rt


def _run_half(rt, xp, result, batch0):
    """Pipeline the NCHUNKS column chunks for this process's core subset.
    xp: [2*n_local, H, Wimg+2*PAD] f16 padded input.  result: either a list of
    6 [N_CORES,1,H,Wimg] f32 arrays (dequantized in the fetch threads, offset
    by batch0) or a [6, n_local, H, Wimg] u8 array (raw wire bytes)."""
    outs = []
    for k in range(NCHUNKS):
        xk = np.ascontiguousarray(xp[:, :, k * S:k * S + W])
        xk_dev = jax.device_put(xk, rt.x_sharding)
        (ok,) = rt.sharded(xk_dev, *rt.band_dev, rt.mask_dev[k], *rt.ghosts)
        outs.append(ok)

    inv_scale = np.float32(1.0 / OSCALE)
    to_f32 = isinstance(result, list)

    def _fetch(job):
        k, shard = job
        c = shard.index[0].start // 6
        u = np.asarray(shard.data)              # [6, H, S] uint8
        for i in range(6):
            if to_f32:
                np.multiply(u[i], inv_scale,
                            out=result[i][batch0 + c, 0][:, k * S:(k + 1) * S],
                            casting="unsafe")
            else:
                result[i, c, :, k * S:(k + 1) * S] = u[i]

    jobs = [(k, sh) for k, o in enumerate(outs) for sh in o.addressable_shards]
    with ThreadPoolExecutor(16) as ex:
        list(ex.map(_fetch, jobs))


def _worker_entry(addr):
    """Second-process entry: owns cores [dev_lo, dev_hi), doubling the axon
    tunnel throughput (two gRPC clients ~55 MB/s vs ~36 MB/s for one)."""
    from multiprocessing.connection import Client
    from multiprocessing import shared_memory

    conn = Client(addr, "AF_UNIX", authkey=b"asa")
    try:
        dev_lo, dev_hi, shm_in_name, shm_out_name, g1, sxh, syh, hk = conn.recv()
        shm_in = shared_memory.SharedMemory(name=shm_in_name)
        shm_out = shared_memory.SharedMemory(name=shm_out_name)
        n_local = dev_hi - dev_lo
        xin = np.ndarray((2 * n_local, H, Wimg + 2 * PAD), np.float16,
                         buffer=shm_in.buf)
        uout = np.ndarray((6, n_local, H, Wimg), np.uint8, buffer=shm_out.buf)
        rt = _setup(g1, sxh, syh, hk, dev_lo, dev_hi)
        _run_half(rt, np.zeros_like(xin), uout, 0)   # warm exec path
        conn.send("ready")
        import time
        while True:
            msg = conn.recv()
            if msg == "quit":
                break
            t0 = time.time()
            _run_half(rt, xin, uout, 0)
            conn.send(("done", t0, time.time()))
    except Exception as e:  # noqa: BLE001
        try:
            conn.send(("error", repr(e)))
        except Exception:   # noqa: BLE001
            pass
        raise


class _State:
    pass


_CACHE = {}


def _boot(g1, sxh, syh, hk):
    """Parent setup: compile + claim cores 0..3, spawn the worker for 4..7."""
    import os
    import shutil
    import subprocess
    import sys
    import uuid
    from multiprocessing.connection import Listener
    from multiprocessing import shared_memory

    st = _State()
    half = N_CORES // 2
    st.half = half
    st.rt = _setup(g1, sxh, syh, hk, 0, half)

    try:
        tagname = uuid.uuid4().hex[:8]
        n_rem = N_CORES - half
        st.shm_in = shared_memory.SharedMemory(
            create=True, name=f"asa_in_{tagname}",
            size=2 * n_rem * H * (Wimg + 2 * PAD) * 2)
        st.shm_out = shared_memory.SharedMemory(
            create=True, name=f"asa_out_{tagname}",
            size=6 * n_rem * H * Wimg)
        st.xin = np.ndarray((2 * n_rem, H, Wimg + 2 * PAD), np.float16,
                            buffer=st.shm_in.buf)
        st.xin[:] = 0
        st.uout = np.ndarray((6, n_rem, H, Wimg), np.uint8,
                             buffer=st.shm_out.buf)

        addr = f"/tmp/asa_sock_{tagname}"
        listener = Listener(addr, "AF_UNIX", authkey=b"asa")
        here = os.path.dirname(os.path.abspath(__file__))
        code = (f"import sys; sys.path.insert(0, {here!r}); "
                f"import kernel; kernel._worker_entry({addr!r})")
        py = shutil.which("python") or sys.executable
        st.proc = subprocess.Popen([py, "-c", code], env=os.environ.copy(),
                                   cwd=here)
        listener._listener._socket.settimeout(120)
        st.conn = listener.accept()
        listener.close()
        try:
            os.unlink(addr)
        except OSError:
            pass
        st.conn.send((half, N_CORES, st.shm_in.name, st.shm_out.name,
                      g1, sxh, syh, hk))
        if not st.conn.poll(900):
            raise RuntimeError("worker setup timeout")
        msg = st.conn.recv()
        if msg != "ready":
            raise RuntimeError(f"worker setup failed: {msg}")
        st.two_proc = True

        import atexit

        def _cleanup(st=st):
            try:
                st.conn.send("quit")
            except Exception:   # noqa: BLE001
                pass
            try:
                st.proc.wait(timeout=2)
            except Exception:   # noqa: BLE001
                try:
                    st.proc.kill()
                except Exception:   # noqa: BLE001
                    pass
            for shm in (st.shm_in, st.shm_out):
                try:
                    shm.close()
                    shm.unlink()
                except Exception:   # noqa: BLE001
                    pass

        atexit.register(_cleanup)
    except Exception:   # noqa: BLE001
        # fall back to single-process on all cores
        try:
            if getattr(st, "proc", None) is not None:
                st.proc.kill()
        except Exception:   # noqa: BLE001
            pass
        st.two_proc = False
        st.rt = _setup(g1, sxh, syh, hk, 0, N_CORES)
        st.half = N_CORES

    # prime the parent exec path once (first real call shouldn't pay warmup)
    xp_warm = np.zeros((2 * st.half, H, Wimg + 2 * PAD), np.float16)
    warm_res = [np.empty((N_CORES, 1, H, Wimg), np.float32) for _ in range(6)]
    _run_half(st.rt, xp_warm, warm_res, 0)
    st.xp_par = xp_warm
    return st


def kernel(spectrogram, gaussian_kernel, sobel_x, sobel_y, harmonic_kernel):
    gk = np.asarray(gaussian_kernel, np.float32).reshape(5, 5)
    sx = np.asarray(sobel_x, np.float32).reshape(3, 3)
    sy = np.asarray(sobel_y, np.float32).reshape(3, 3)
    hk = np.asarray(harmonic_kernel, np.float32).reshape(7)
    g1 = (gk[2] / gk[2].sum()).astype(np.float32)
    sxh = sx[:, 2].astype(np.float32)           # [1,2,1]/8
    syh = (sy[:, 1] / 2.0).astype(np.float32)   # [-1,0,1]/8

    if "st" not in _CACHE:
        _CACHE["st"] = _boot(g1, sxh, syh, hk)
    st = _CACHE["st"]

    sp = np.asarray(spectrogram).reshape(2 * N_CORES, H, Wimg)
    result = [np.empty((N_CORES, 1, H, Wimg), np.float32) for _ in range(6)]

    if st.two_proc:
        # hand batches [half, 8) to the worker first so both tunnels stream
        np.copyto(st.xin[:, :, PAD:PAD + Wimg], sp[2 * st.half:],
                  casting="same_kind")
        st.conn.send("go")

    if _DEBUG:
        import time as _t
        _t0 = _t.time()
    np.copyto(st.xp_par[:, :, PAD:PAD + Wimg], sp[:2 * st.half],
              casting="same_kind")
    _run_half(st.rt, st.xp_par, result, 0)
    if _DEBUG:
        print(f"[par] own half {_t0:.3f}..{_t.time():.3f} "
              f"({_t.time()-_t0:.3f}s)", flush=True)

    if st.two_proc:
        if not st.conn.poll(300):
            raise RuntimeError("worker timeout")
        msg = st.conn.recv()
        if not (isinstance(msg, tuple) and msg[0] == "done"):
            raise RuntimeError(f"worker failed: {msg}")
        if _DEBUG:
            import time as _t
            print(f"[par] worker ran {msg[1]:.3f}..{msg[2]:.3f} "
                  f"({msg[2]-msg[1]:.3f}s), parent now {_t.time():.3f}",
                  flush=True)
        inv_scale = np.float32(1.0 / OSCALE)
        for i in range(6):
            for c in range(N_CORES - st.half):
                np.multiply(st.uout[i, c], inv_scale,
                            out=result[i][st.half + c, 0], casting="unsafe")

    return tuple(result)
